# revision 29
# baseline (speedup 1.0000x reference)
"""LIF spiking-neuron recurrence kernel for Trainium2 (Bass/Tile, 8-core SPMD).

Problem: x [32, 128, 32, 32, 8] f32, time on the LAST axis (T=8).
    u_0 = x_0;  o_t = (u_t > Vth);  u_{t+1} = TAU * u_t * (1 - o_t) + x_{t+1}
Output: spikes o [32, 128, 32, 32, 8] f32 (0.0 / 1.0).

Sharding: pure data-parallel over the batch dim (32 -> 4 per core, 8 cores).

Memory-roofline design (HBM traffic 32 -> ~9.7 MB/core):
  - x converted to fp16 on host (load 8.4 MB/core). Simulated rel err vs
    the f32 reference is 4.6e-3, well under the 2e-2 gate.
  - Spike masks leave the device bit-packed: the PE accumulates scaled
    identity matmuls into two nibble planes P_lo = sum_{t<4} 2^t m_t and
    P_hi = sum_{t>=4} 2^(t-4) m_t (integers 0..15, exact in fp8e4), so
    the store is 2 x 0.5 MB/core. Host decodes bits and emits o = 1 - m.

Per-timestep compute (on [128, 4096] fp16 planes):
    m'  = (u <= Vth) * TAU            DVE tensor_scalar is_le+mult, 4x mode
    v   = u * m'                      DVE tensor_tensor mult, 2x mode
    u   = v + x_{t+1}                 DVE tensor_tensor add, 2x mode
    PSUM += (2^(t%4+2) I) @ m'        PE, accumulates 2^(t%4) * m_t exactly
                                      (banks reused: group t<4, group t>=4)
    ACT copies each finished PSUM bank to SBUF fp8; HWDGE stores.

The fused is_le+mult keeps the compare in the DVE 4x path (the
scalar_tensor_tensor form measured at 1x = 4.4us/plane); TAU = 2^-2 so
all scales are exact powers of two and device arithmetic stays
bit-identical to an fp16 numpy simulation of the recurrence.
"""

import numpy as np

import bass_rust
import concourse.bass as bass
import concourse.mybir as mybir
import concourse.tile as tile
from concourse.bass_utils import run_bass_kernel_spmd

VTH = 0.2
TAU = 0.25

N_CORES = 8
FULL_SHAPE = (32, 128, 32, 32, 8)
B_PER_CORE = FULL_SHAPE[0] // N_CORES  # 4
T = FULL_SHAPE[-1]  # 8
T_PE = 4  # planes packed by the PE; the rest go out as fp8 directly

PIX = B_PER_CORE * FULL_SHAPE[1] * FULL_SHAPE[2] * FULL_SHAPE[3]  # 524288
P_DIM = 128
C = PIX // P_DIM  # 4096 pixels per partition
BANK = 512  # PSUM bank free size (fp32)
NBANK = C // BANK  # 8

_cache: dict = {}


def _split_multi_waits(nc: bass.Bass) -> int:
    """Hoist all-but-one embedded sync waits onto standalone EventSemaphore
    instructions. The walrus build behind bass2jax rejects >1 sync wait per
    instruction ("Too many sync wait commands"); a standalone wait on the
    same engine stream immediately before is semantically identical."""
    n = 0
    for fn in nc.m.functions:
        for block in fn.blocks:
            out = []
            changed = False
            for ins in block.instructions:
                si = ins.sync_info
                waits = list(si.on_wait) if si is not None else []
                if len(waits) > 1:
                    for k, w in enumerate(waits[:-1]):
                        ev = mybir.InstEventSemaphore(
                            name=f"{ins.name}-hw{k}", ins=[], outs=[]
                        )
                        ev.sync_info = bass_rust.SyncInfo(
                            on_wait=[w], on_update=[]
                        )
                        ev.engine = ins.engine
                        nc.inst_map[ev.name] = ev
                        out.append(ev)
                        n += 1
                    si.on_wait = [waits[-1]]
                    changed = True
                out.append(ins)
            if changed:
                block.instructions = out
    return n


def _build_bass() -> bass.Bass:
    f16 = mybir.dt.float16
    f32 = mybir.dt.float32
    f8 = mybir.dt.float8e4
    Alu = mybir.AluOpType
    Act = mybir.ActivationFunctionType

    nc = bass.Bass(trn_type="TRN2")
    x_d = nc.dram_tensor("x", [P_DIM, T * C], f16, kind="ExternalInput")
    wp_d = nc.dram_tensor("wp", [P_DIM, (T - 1) * P_DIM], f16, kind="ExternalInput")
    # y[:, 0:C] = P_lo (bits t=0..3); y[:, C:2C] = P_hi (bits t=4..6);
    # y[:, 2C:3C] = m'_7 raw (nonzero means m=1)
    y_d = nc.dram_tensor("y", [P_DIM, 3 * C], f8, kind="ExternalOutput")

    with tile.TileContext(nc) as tc:
        with (
            tc.tile_pool(name="px", bufs=8) as px,
            tc.tile_pool(name="pw", bufs=1) as pw,
            tc.tile_pool(name="pm", bufs=4) as pm,
            tc.tile_pool(name="pst", bufs=1) as pst,
            tc.tile_pool(name="pout", bufs=3) as pout,
            tc.tile_pool(name="ppsum", bufs=1, space="PSUM") as ppsum,
        ):
            # pack weights ride the otherwise-idle scalar HWDGE queue so
            # they land in parallel with x plane 0 (PE needs them first)
            wp = pw.tile([P_DIM, (T - 1) * P_DIM], f16, tag="wp")
            nc.scalar.dma_start(wp, wp_d[:, :])
            # x planes, t-plane-major; plane 0 loads as four quarter-planes
            # so step-0 compute starts as soon as the first 0.26 MB lands.
            H = C // 2
            Q = C // 4
            xh = []
            for q in range(4):
                p = px.tile([P_DIM, Q], f16, tag="xph", name=f"xph{q}")
                nc.sync.dma_start(p, x_d[:, q * Q : (q + 1) * Q])
                xh.append(p)
            xp = [None]
            for t in range(1, T):
                p = px.tile([P_DIM, C], f16, tag="xp", name=f"xp{t}")
                nc.sync.dma_start(p, x_d[:, t * C : (t + 1) * C])
                xp.append(p)

            banks = [
                ppsum.tile([P_DIM, BANK], f32, tag=f"bank{b}", name=f"bank{b}")
                for b in range(NBANK)
            ]

            u = pst.tile([P_DIM, C], f16, tag="u")
            v = pst.tile([P_DIM, C], f16, tag="v")

            # step 0 cmp/mult run per quarter-plane (u_0 = x_0), so the DVE
            # starts as soon as the first quarter of plane 0 lands
            m0 = pm.tile([P_DIM, C], f16, tag="m", name="m0")
            for q in range(4):
                cols = slice(q * Q, (q + 1) * Q)
                nc.vector.tensor_scalar(
                    m0[:, cols], xh[q], VTH, TAU, Alu.is_le, Alu.mult
                )
                for j in range(NBANK // 4):
                    b = q * (NBANK // 4) + j
                    nc.tensor.matmul(
                        banks[b],
                        wp[:, 0:P_DIM],
                        m0[:, b * BANK : (b + 1) * BANK],
                        start=True,
                        stop=False,
                    )
                nc.vector.tensor_tensor(v[:, cols], xh[q], m0[:, cols], Alu.mult)
            nc.vector.tensor_tensor(u, v, xp[1], Alu.add)

            for t in range(1, T - 2):
                m = pm.tile([P_DIM, C], f16, tag="m", name=f"m{t}")
                # m' = (u <= Vth) * TAU in {0, TAU}; 4x DVE mode
                nc.vector.tensor_scalar(m, u, VTH, TAU, Alu.is_le, Alu.mult)
                # PSUM bank b += 2^(t%4+2) * m'[:, bank b]  (= 2^(t%4) * m_t)
                lhs = wp[:, t * P_DIM : (t + 1) * P_DIM]
                for b in range(NBANK):
                    nc.tensor.matmul(
                        banks[b],
                        lhs,
                        m[:, b * BANK : (b + 1) * BANK],
                        start=(t % T_PE == 0),
                        stop=(t % T_PE == T_PE - 1 or t == T - 2),
                    )
                if t % T_PE == T_PE - 1 or t == T - 2:
                    # nibble plane (exact small ints in fp8e4) -> SBUF -> HBM
                    half = t // T_PE
                    pk = pout.tile([P_DIM, C], f8, tag="pk", name=f"pk{half}")
                    for b in range(NBANK):
                        cols = slice(b * BANK, (b + 1) * BANK)
                        nc.scalar.activation(pk[:, cols], banks[b], Act.Copy)
                    nc.sync.dma_start(y_d[:, half * C : (half + 1) * C], pk)
                # v = u * m'; u = v + x_{t+1}; both 2x DVE mode
                nc.vector.tensor_tensor(v, u, m, Alu.mult)
                nc.vector.tensor_tensor(u, v, xp[t + 1], Alu.add)

            # t = T-2: last state update, mult/add chunked per half so the
            # final mask (computed directly in fp8; tensor_scalar keeps
            # 2x_2P with an 8-bit out) and its store overlap the update.
            t = T - 2
            m = pm.tile([P_DIM, C], f16, tag="m", name=f"m{t}")
            nc.vector.tensor_scalar(m, u, VTH, TAU, Alu.is_le, Alu.mult)
            lhs = wp[:, t * P_DIM : (t + 1) * P_DIM]
            for b in range(NBANK):
                nc.tensor.matmul(
                    banks[b],
                    lhs,
                    m[:, b * BANK : (b + 1) * BANK],
                    start=(t % T_PE == 0),
                    stop=True,
                )
            pk = pout.tile([P_DIM, C], f8, tag="pk", name="pk1")
            for b in range(NBANK):
                cols = slice(b * BANK, (b + 1) * BANK)
                nc.scalar.activation(pk[:, cols], banks[b], Act.Copy)
            nc.sync.dma_start(y_d[:, C : 2 * C], pk)
            for h in range(2):
                cols = slice(h * H, (h + 1) * H)
                nc.vector.tensor_tensor(v[:, cols], u[:, cols], m[:, cols], Alu.mult)
                nc.vector.tensor_tensor(u[:, cols], v[:, cols], xp[T - 1][:, cols], Alu.add)
                m7 = pout.tile([P_DIM, H], f8, tag="m7", name=f"m7_{h}")
                nc.vector.tensor_scalar(m7, u[:, cols], VTH, TAU, Alu.is_le, Alu.mult)
                nc.sync.dma_start(y_d[:, 2 * C + h * H : 2 * C + (h + 1) * H], m7)

    _split_multi_waits(nc)
    return nc


def _pack_weights() -> np.ndarray:
    wp = np.zeros((P_DIM, (T - 1) * P_DIM), dtype=np.float16)
    for t in range(T - 1):
        wp[:, t * P_DIM : (t + 1) * P_DIM] = np.eye(P_DIM, dtype=np.float16) * (
            2.0 ** (t % T_PE + 2)
        )
    return wp


def _shard(x16: np.ndarray, c: int) -> np.ndarray:
    """Core c's shard, t-plane-major fp16: [PIX, T] -> [128, T, C] -> flat."""
    s = x16[c * B_PER_CORE : (c + 1) * B_PER_CORE].reshape(P_DIM, C, T)
    return np.ascontiguousarray(s.transpose(0, 2, 1)).reshape(P_DIM, T * C)


def _unshard(y: np.ndarray) -> np.ndarray:
    """Decode one core's output -> spikes o [4, 128, 32, 32, 8]."""
    y = np.asarray(y).astype(np.float32)  # fp8 -> f32 (exact small values)
    m = np.empty((P_DIM, C, T), dtype=np.uint8)
    for half in range(2):
        pk = y[:, half * C : (half + 1) * C].astype(np.uint8)
        m[:, :, half * T_PE : (half + 1) * T_PE] = np.unpackbits(
            pk[..., None], axis=-1, count=T_PE, bitorder="little"
        )
    m[:, :, T - 1] = y[:, 2 * C : 3 * C] != 0.0  # raw m'_7 plane
    o = (1 - m).astype(np.float32)
    return o.reshape(B_PER_CORE, *FULL_SHAPE[1:])


def kernel(x: np.ndarray) -> np.ndarray:
    assert x.shape == FULL_SHAPE, x.shape
    in_dtype = x.dtype

    if "nc" not in _cache:
        _cache["nc"] = _build_bass()
        _cache["wp"] = _pack_weights()
    nc = _cache["nc"]
    wp = _cache["wp"]

    x16 = np.asarray(x, dtype=np.float16)
    in_maps = [{"x": _shard(x16, c), "wp": wp} for c in range(N_CORES)]
    res = run_bass_kernel_spmd(nc, in_maps, core_ids=list(range(N_CORES)))
    out = np.concatenate(
        [_unshard(res.results[c]["y"]) for c in range(N_CORES)], axis=0
    )
    return out.astype(in_dtype, copy=False)
